# revision 1
# baseline (speedup 1.0000x reference)
"""Trainium2 Bass kernel for CustomSelfAttentionWithBias (B=2, T=2048, C=1024, H=16).

Computes y = proj(softmax(mask(QK^T/sqrt(hd) + emphasis_col0)) @ V) where
qkv = x @ W_attn, with a causal bool mask and +1.0 emphasis on score column 0.

Sharding: 8 cores; core c handles batch b = c//4 and heads 4*(c%4) .. +4
(data parallel on B, tensor parallel on heads; c_proj row-sharded so each
core emits a partial y[b] that the host sums).

v2 design notes (vs the 232us baseline):
  - The PE drops to 1.2GHz after ANY idle gap and only re-reaches 2.4GHz
    after 3us of continuous work, so the whole kernel is emitted as one
    gapless PE stream: qkv-generation groups and projection groups are
    woven as *filler* work between attention chunks instead of running as
    separate phases.
  - Startup: host pre-arranges x[b].T into [128, 8, 2048]; weights are
    DMA'd first and xT arrives in 4 t-block chunks so the first matmul
    starts ~10us earlier.
  - Causal narrowing: diagonal key-chunks only compute the q >= 128*r
    columns in scores/exp/PV; masking shrinks to one 128x128 triangle
    multiply per head per diagonal chunk.
  - Norm chain: reciprocal_approx_fast reads the PSUM denominator row
    directly, one broadcast DMA per head, muls split across Pool+DVE.
  - PSUM: 2x[128,1024] score tiles + 4x[128,512] shared work tiles
    (PV accumulators, gen groups, proj groups) = exactly 8 banks.
"""

import math
import numpy as np
import ml_dtypes

B, T, C = 2, 2048, 1024
H, HD = 16, 64
NH = 4            # heads per core
N_CORES = 8
QB = 512          # query block (columns of S^T per matmul)
KC = 128          # key chunk (partition dim of S^T)
N_QB = T // QB    # 4
N_KC = T // KC    # 16
CCH = C // 128    # 8 contraction chunks for the projections
EMPHASIS = 1.0
PEND = 3          # PV pending depth (chunks between QK and PV emission)
BISECT_NOFILL = False  # fillers woven between attention chunks

_COMPILED = {}


def _build(causal: bool = True):
    import concourse.bass as bass
    import concourse.tile as tile
    import concourse.mybir as mybir
    from concourse import bacc

    f32 = mybir.dt.float32
    f16 = mybir.dt.float16
    bf16 = mybir.dt.bfloat16
    EXP = mybir.ActivationFunctionType.Exp

    nc = bacc.Bacc("TRN2", target_bir_lowering=False, debug=False)

    xtr = nc.dram_tensor("xtr", [128, CCH, T], bf16, kind="ExternalInput").ap()
    wq = nc.dram_tensor("wq", [C, NH * HD], bf16, kind="ExternalInput").ap()
    wk = nc.dram_tensor("wk", [C, NH * HD], bf16, kind="ExternalInput").ap()
    wv = nc.dram_tensor("wv", [C, NH * HD], bf16, kind="ExternalInput").ap()
    wp = nc.dram_tensor("wp", [NH * HD, C], bf16, kind="ExternalInput").ap()
    tri = nc.dram_tensor("tri", [128, 128], bf16, kind="ExternalInput").ap()
    y = nc.dram_tensor("y", [T, C], f16, kind="ExternalOutput").ap()

    with tile.TileContext(nc) as tc:
        _body(nc, tc, bass, mybir, xtr, wq, wk, wv, wp, tri, y, causal,
              f32, f16, bf16, EXP)
    nc.compile()
    return nc


def _body(nc, tc, bass, mybir, xtr, wq, wk, wv, wp, tri, y, causal,
          f32, f16, bf16, EXP):
    from contextlib import ExitStack

    ctx = ExitStack()
    singles = ctx.enter_context(tc.tile_pool(name="singles", bufs=1))
    # scores + wide filler groups (gen/proj): all short-lived, 2x2 banks
    ps_st = ctx.enter_context(tc.tile_pool(name="ps_st", bufs=2, space="PSUM"))
    # PV accumulator pairs (long-lived): dedicated 4x1 banks
    ps_po = ctx.enter_context(tc.tile_pool(name="ps_po", bufs=4, space="PSUM"))
    pt_pool = ctx.enter_context(tc.tile_pool(name="pt_pool", bufs=PEND + 2))
    rec_pool = ctx.enter_context(tc.tile_pool(name="rec_pool", bufs=4))
    rs_pool = ctx.enter_context(tc.tile_pool(name="rs_pool", bufs=2))
    osh_pool = ctx.enter_context(tc.tile_pool(name="osh_pool", bufs=2))
    y_pool = ctx.enter_context(tc.tile_pool(name="y_pool", bufs=3))

    # ---- resident SBUF tiles --------------------------------------------
    wq_sb = singles.tile([128, CCH, NH * HD], bf16, name="wq_sb")
    wk_sb = singles.tile([128, CCH, NH * HD], bf16, name="wk_sb")
    wv_sb = singles.tile([128, CCH, NH * HD], bf16, name="wv_sb")
    wp_sb = singles.tile([128, 2, C], bf16, name="wp_sb")
    tri_sb = singles.tile([128, 128], bf16, name="tri_sb")
    # xT per t-block: [128, cc, 512]
    xt_t = [singles.tile([128, CCH, QB], bf16, name=f"xt{i}")
            for i in range(N_QB)]
    # Q^T / K^T per (head pair, t-block): [128 = 2 heads x 64, 512]
    qt_t = [[singles.tile([128, QB], bf16, name=f"qt{p}_{i}")
             for i in range(N_QB)] for p in range(2)]
    kt_t = [[singles.tile([128, QB], bf16, name=f"kt{p}_{i}")
             for i in range(N_QB)] for p in range(2)]
    # V|ones per kc pair: [128 k, 2, head, 65]
    v_t = [singles.tile([128, 2, NH, HD + 1], bf16, name=f"v{j}")
           for j in range(N_KC // 2)]
    # O^T per (head pair, q-block)
    ot_t = [[singles.tile([128, QB], bf16, name=f"ot{p}_{i}")
             for i in range(N_QB)] for p in range(2)]

    # ---- input DMAs (weights first, then xT in t-block chunks) ----------
    nc.sync.dma_start(out=wq_sb, in_=wq.rearrange("(c p) n -> p c n", p=128))
    nc.sync.dma_start(out=wk_sb, in_=wk.rearrange("(c p) n -> p c n", p=128))
    nc.sync.dma_start(out=wv_sb, in_=wv.rearrange("(c p) n -> p c n", p=128))
    nc.sync.dma_start(out=wp_sb, in_=wp.rearrange("(j p) n -> p j n", p=128))
    nc.sync.dma_start(out=tri_sb, in_=tri)
    for i in range(N_QB):
        nc.sync.dma_start(out=xt_t[i], in_=xtr[:, :, i * QB:(i + 1) * QB])
    for j in range(N_KC // 2):
        nc.vector.memset(v_t[j][:, :, :, HD:HD + 1], 1.0)

    # ---- group emitters (each = one PSUM-tile's worth of matmuls) ------
    def cast(eng, out, in_):
        if eng is nc.scalar:
            eng.copy(out, in_)
        else:
            eng.tensor_copy(out, in_)

    def gen_qkw(dst, w_sb, nb, eng):
        # wide: Q^T (or K^T) for BOTH head pairs of t-block nb
        pg = ps_st.tile([128, 2 * QB], f32, tag="st", name="pg_qk")
        for pr in range(2):
            for cc in range(CCH):
                nc.tensor.matmul(
                    pg[:, pr * QB:(pr + 1) * QB],
                    w_sb[:, cc, pr * 128:(pr + 1) * 128],
                    xt_t[nb][:, cc, :],
                    start=(cc == 0), stop=(cc == CCH - 1))
        for pr in range(2):
            cast(eng, dst[pr][nb], pg[:, pr * QB:(pr + 1) * QB])

    def gen_vw(nb, eng):
        # wide: V|ones for the 4 key chunks of t-block nb
        pg = ps_st.tile([128, 2 * QB], f32, tag="st", name="pg_v")
        for q in range(4):
            for cc in range(CCH):
                nc.tensor.matmul(
                    pg[:, q * 256:(q + 1) * 256],
                    xt_t[nb][:, cc, q * 128:(q + 1) * 128],
                    wv_sb[:, cc, :],
                    start=(cc == 0), stop=(cc == CCH - 1))
        for q in range(4):
            cast(
                eng, v_t[2 * nb + q // 2][:, q % 2, :, 0:HD],
                pg[:, q * 256:(q + 1) * 256].rearrange(
                    "p (h n) -> p h n", h=NH))
        if nb == 0:
            # emphasis: P column for k==0 gets exp(s+1); fold e into V|ones
            nc.scalar.mul(v_t[0][0:1, 0, :, :], v_t[0][0:1, 0, :, :],
                          float(math.exp(EMPHASIS)))

    def proj_w(qb, tci, eng):
        # wide: both 512-col halves of y rows [128*(4qb+tci) ..]
        tc_i = 4 * qb + tci
        py = ps_st.tile([128, 2 * QB], f32, tag="st", name="py_w")
        for ch in range(2):
            for pr2 in range(2):
                nc.tensor.matmul(
                    py[:, ch * QB:(ch + 1) * QB],
                    ot_t[pr2][qb][:, tci * 128:(tci + 1) * 128],
                    wp_sb[:, pr2, ch * QB:(ch + 1) * QB],
                    start=(pr2 == 0), stop=(pr2 == 1))
        ysb = y_pool.tile([128, C], f16, tag="ysb", name=f"ysb{tc_i}")
        cast(eng, ysb, py)
        nc.sync.dma_start(out=y[tc_i * 128:(tc_i + 1) * 128, :], in_=ysb)

    def proj_tail_wave(qb, tcis, eng0, eng1):
        # narrow groups from the (now idle) po pool; all pr2=0 matmuls are
        # emitted first so they run while the last head-pair is still being
        # normalized
        pys = {}
        for tci in tcis:
            for ch in range(2):
                py = pys[(tci, ch)] = ps_po.tile(
                    [128, QB], f32, tag="po", name=f"py_{tci}_{ch}")
                nc.tensor.matmul(
                    py, ot_t[0][qb][:, tci * 128:(tci + 1) * 128],
                    wp_sb[:, 0, ch * QB:(ch + 1) * QB],
                    start=True, stop=False)
        for tci in tcis:
            tc_i = 4 * qb + tci
            ysb = y_pool.tile([128, C], f16, tag="ysb", name=f"ysb{tc_i}")
            for ch, eng in ((0, eng0), (1, eng1)):
                nc.tensor.matmul(
                    pys[(tci, ch)],
                    ot_t[1][qb][:, tci * 128:(tci + 1) * 128],
                    wp_sb[:, 1, ch * QB:(ch + 1) * QB],
                    start=False, stop=True)
                cast(eng, ysb[:, ch * QB:(ch + 1) * QB], pys[(tci, ch)])
            nc.sync.dma_start(out=y[tc_i * 128:(tc_i + 1) * 128, :], in_=ysb)

    def gen_groups(nb):
        return [(gen_qkw, qt_t, wq_sb, nb), (gen_qkw, kt_t, wk_sb, nb),
                (gen_vw, nb)]

    def proj_groups(qb):
        return [(proj_w, qb, tci) for tci in range(4)]

    def run_group(g, eng):
        if g[0] is gen_qkw:
            gen_qkw(g[1], g[2], g[3], eng)
        elif g[0] is gen_vw:
            gen_vw(g[1], eng)
        else:
            proj_w(g[1], g[2], eng)

    # ---- attention ------------------------------------------------------
    def norm(pr, qb, s, po, last):
        # den row PSUM->SBUF, broadcast, then 1/den in place on DVE (the
        # custom-DVE write is only read by the same-engine mul: custom-DVE
        # writes are not cross-engine tracked), then multiply.
        rs = rs_pool.tile([HD + 1, QB], f32, tag="rs")
        nc.vector.tensor_copy(rs[HD:HD + 1, :], po[HD:HD + 1, :])
        rec = rec_pool.tile([HD, QB], f32, tag="rec")
        nc.sync.dma_start(
            out=rec,
            in_=rs[HD:HD + 1, :].unsqueeze(1).broadcast_to([1, HD, QB]))
        nc.vector.reciprocal_approx_fast(out=rec, in_=rec)
        if s == 0:
            nc.vector.tensor_mul(ot_t[pr][qb][0:HD, :], po[0:HD, :], rec)
        else:
            osh = osh_pool.tile([HD, QB], bf16, tag="osh")
            nc.vector.tensor_mul(osh, po[0:HD, :], rec)
            nc.sync.dma_start(out=ot_t[pr][qb][HD:128, :], in_=osh)

    def emit_pv(rec_):
        pr, qb, kc, w0, pt, po0, po1, nk = rec_
        v = v_t[kc // 2][:, kc % 2, :, :]
        nc.tensor.matmul(po0[0:HD + 1, w0:QB], v[:, 2 * pr, :],
                         pt[:, w0:QB],
                         start=(kc == 0), stop=(kc == nk - 1))
        nc.tensor.matmul(po1[0:HD + 1, w0:QB], v[:, 2 * pr + 1, :],
                         pt[:, QB + w0:2 * QB],
                         start=(kc == 0), stop=(kc == nk - 1))
        if kc == nk - 1:
            last = (qb == N_QB - 1)
            norm(pr, qb, 1, po1, last)
            norm(pr, qb, 0, po0, last)

    pending = []
    fillers = []
    f_emitted = 0

    def pace_fillers(i, nq, total):
        nonlocal f_emitted
        # front-load gen fillers (every other chunk from the start); proj
        # fillers spread out after the prev block's norms are emitted
        while f_emitted < total and fillers:
            g = fillers[0]
            is_proj = g[0] is proj_w
            due = sched_proj if is_proj else sched_gen
            if not due or i < due[0]:
                break
            due.pop(0)
            fillers.pop(0)
            run_group(g, filler_eng)
            f_emitted += 1

    # gen(0) runs as the prefix; casts on the then-idle ACT engine
    gen_qkw(qt_t, wq_sb, 0, nc.vector)
    gen_qkw(kt_t, wk_sb, 0, nc.vector)
    gen_vw(0, nc.vector)

    for qb in range(N_QB):
        nk = 4 * (qb + 1) if causal else N_KC
        nq = 2 * nk
        fillers = []
        if qb + 1 < N_QB:
            fillers += gen_groups(qb + 1)
        if qb >= 1:
            fillers += proj_groups(qb - 1)
        total = len(fillers)
        filler_eng = nc.vector
        n_gen = sum(1 for g in fillers if g[0] is not proj_w)
        n_proj = total - n_gen
        sched_gen = {0: [0, 1, 5], 1: [0, 2, 4], 2: [0, 3, 6], 3: []}[qb][:n_gen]
        sched_proj = {0: [], 1: [6, 9, 12, 15], 2: [8, 12, 16, 20],
                      3: [6, 12, 18, 24]}[qb][:n_proj]
        f_emitted = 0
        i = 0
        for pr in range(2):
            po0 = ps_po.tile([128, QB], f32, tag="po", name="po0")
            po1 = ps_po.tile([128, QB], f32, tag="po", name="po1")
            for kc in range(nk):
                r = kc - 4 * qb
                w0 = 128 * r if (causal and r > 0) else 0
                st = ps_st.tile([128, 2 * QB], f32, tag="st")
                for s in range(2):
                    r0, r1 = s * HD, (s + 1) * HD
                    nc.tensor.matmul(
                        st[:, s * QB + w0:(s + 1) * QB],
                        kt_t[pr][kc // 4][r0:r1, (kc % 4) * 128:(kc % 4 + 1) * 128],
                        qt_t[pr][qb][r0:r1, w0:QB],
                        start=True, stop=True)
                pt = pt_pool.tile([128, 2 * QB], bf16, tag="pt")
                if w0 == 0:
                    nc.scalar.activation(out=pt, in_=st, func=EXP)
                else:
                    stv = st.rearrange("p (a q) -> p a q", a=2)
                    ptv = pt.rearrange("p (a q) -> p a q", a=2)
                    nc.scalar.activation(out=ptv[:, :, w0:QB],
                                         in_=stv[:, :, w0:QB], func=EXP)
                if causal and r >= 0:
                    for s, meng in ((0, nc.vector), (1, nc.vector)):
                        meng.tensor_mul(
                            pt[:, s * QB + w0:s * QB + w0 + 128],
                            pt[:, s * QB + w0:s * QB + w0 + 128],
                            tri_sb)
                pending.append((pr, qb, kc, w0, pt, po0, po1, nk))
                while len(pending) > PEND:
                    emit_pv(pending.pop(0))
                if not BISECT_NOFILL:
                    pace_fillers(i, nq, total)
                i += 1
        if BISECT_NOFILL:
            while pending:
                emit_pv(pending.pop(0))
            while fillers:
                g = fillers.pop(0)
                run_group(g, filler_eng)
    while pending:
        emit_pv(pending.pop(0))
    # trailing projection of the last q-block: narrow groups from the now
    # idle po pool, casts alternating engines so consecutive groups pipeline
    for tcis in ((0, 1), (2, 3)):
        proj_tail_wave(N_QB - 1, tcis, nc.scalar, nc.vector)

    ctx.close()


def _prep_inputs(x, W_attn, W_proj, attn_mask):
    """Host-side shard + layout prep. Returns (in_maps, causal)."""
    bf = ml_dtypes.bfloat16
    causal = bool(np.array_equal(
        np.asarray(attn_mask),
        np.tril(np.ones((T, T), dtype=bool))))

    x = np.asarray(x, dtype=np.float32)
    Wa = np.asarray(W_attn, dtype=np.float32)
    Wp = np.asarray(W_proj, dtype=np.float32)

    scale = 1.0 / np.sqrt(np.float32(HD))
    # [128, cc, T]: partition p holds rows c = cc*128 + p of x[b].T
    xtr_b = [np.ascontiguousarray(
        x[b].T.reshape(CCH, 128, T).transpose(1, 0, 2)).astype(bf)
        for b in range(B)]

    # in-stripe causal triangle: tri[k, q] = 1.0 if k <= q else 0
    i = np.arange(128)
    tri = (i[:, None] <= i[None, :]).astype(bf)

    in_maps = []
    for core in range(N_CORES):
        b, h0 = core // 4, (core % 4) * NH
        hsl = slice(h0 * HD, (h0 + NH) * HD)
        wq_c = np.ascontiguousarray(Wa[:, hsl] * scale).astype(bf)
        wk_c = np.ascontiguousarray(Wa[:, C + h0 * HD: C + (h0 + NH) * HD]).astype(bf)
        wv_c = np.ascontiguousarray(Wa[:, 2 * C + h0 * HD: 2 * C + (h0 + NH) * HD]).astype(bf)
        wp_c = np.ascontiguousarray(Wp[hsl, :]).astype(bf)
        in_maps.append({
            "xtr": xtr_b[b], "wq": wq_c, "wk": wk_c, "wv": wv_c,
            "wp": wp_c, "tri": tri,
        })
    return in_maps, causal


def kernel(x, W_attn, W_proj, attn_mask, _trace=False):
    from concourse import bass_utils

    in_maps, causal = _prep_inputs(x, W_attn, W_proj, attn_mask)
    key = ("causal" if causal else "dense")
    if key not in _COMPILED:
        _COMPILED[key] = _build(causal)
    nc = _COMPILED[key]

    res = bass_utils.run_bass_kernel_spmd(
        nc, in_maps, core_ids=list(range(N_CORES)), trace=_trace)

    y = np.zeros((B, T, C), dtype=np.float32)
    for core in range(N_CORES):
        y[core // 4] += res.results[core]["y"].astype(np.float32)
    if _trace:
        kernel._last_results = res
    return y



# revision 7
# speedup vs baseline: 1.3796x; 1.3796x over previous
"""Trainium2 Bass kernel for CustomSelfAttentionWithBias (B=2, T=2048, C=1024, H=16).

Computes y = proj(softmax(mask(QK^T/sqrt(hd) + emphasis_col0)) @ V) where
qkv = x @ W_attn, with a causal bool mask and +1.0 emphasis on score column 0.

Sharding: 8 cores; core c handles batch b = c//4 and heads 4*(c%4) .. +4
(data parallel on B, tensor parallel on heads; c_proj row-sharded so each
core emits a partial y[b] that the host sums).

v3 design notes (vs the 233us v2):
  - v2's remaining 76us of PE idle was ~8 pair-boundary stalls: the norm
    chain (den copy -> broadcast DMA -> recip -> mul -> osh DMA) has
    ~2.5-6.5us of fixed DMA latency per hop, and a stalled proj filler in
    the in-order PE queue blocks all later PE work. v3 replaces the
    broadcast DMA with gpsimd partition_broadcast (idle Pool engine, no
    DMA machinery) and moves the den row copy to Pool too.
  - Startup: wq is DMA'd first and xt[0] arrives in 8 per-cc slices so
    the first gen matmul starts at ~3.5us instead of ~11.8us.
  - Causal narrowing: diagonal key-chunks only compute the q >= 128*r
    columns in scores/exp/PV; masking shrinks to one 128x128 triangle
    multiply per head per diagonal chunk.
  - PSUM: 2x[128,1024] score tiles + 4x[128,512] shared work tiles
    (PV accumulators, gen groups, proj groups) = exactly 8 banks.
"""

import math
import numpy as np
import ml_dtypes

B, T, C = 2, 2048, 1024
H, HD = 16, 64
NH = 4            # heads per core
N_CORES = 8
QB = 512          # query block (columns of S^T per matmul)
KC = 128          # key chunk (partition dim of S^T)
N_QB = T // QB    # 4
N_KC = T // KC    # 16
CCH = C // 128    # 8 contraction chunks for the projections
EMPHASIS = 1.0
PEND = 3          # PV pending depth (chunks between QK and PV emission)
BISECT_NOFILL = False  # fillers woven between attention chunks

_COMPILED = {}


def _build(causal: bool = True):
    import concourse.bass as bass
    import concourse.tile as tile
    import concourse.mybir as mybir
    from concourse import bacc
    from concourse import library_config

    f32 = mybir.dt.float32
    f16 = mybir.dt.float16
    bf16 = mybir.dt.bfloat16
    EXP = mybir.ActivationFunctionType.Exp

    nc = bacc.Bacc("TRN2", target_bir_lowering=False, debug=False)

    xtr = nc.dram_tensor("xtr", [128, CCH, T], bf16, kind="ExternalInput").ap()
    wq = nc.dram_tensor("wq", [C, NH * HD], bf16, kind="ExternalInput").ap()
    wk = nc.dram_tensor("wk", [C, NH * HD], bf16, kind="ExternalInput").ap()
    wv = nc.dram_tensor("wv", [C, NH * HD], bf16, kind="ExternalInput").ap()
    wp = nc.dram_tensor("wp", [NH * HD, C], bf16, kind="ExternalInput").ap()
    tri = nc.dram_tensor("tri", [128, 128], bf16, kind="ExternalInput").ap()
    y = nc.dram_tensor("y", [T, C], f16, kind="ExternalOutput").ap()

    with tile.TileContext(nc) as tc:
        _body(nc, tc, bass, mybir, library_config, xtr, wq, wk, wv, wp, tri,
              y, causal, f32, f16, bf16, EXP)
    nc.compile()
    return nc


def _body(nc, tc, bass, mybir, library_config, xtr, wq, wk, wv, wp, tri, y,
          causal, f32, f16, bf16, EXP):
    from contextlib import ExitStack

    ctx = ExitStack()
    singles = ctx.enter_context(tc.tile_pool(name="singles", bufs=1))
    # scores + wide filler groups (gen/proj): all short-lived, 2x2 banks
    ps_st = ctx.enter_context(tc.tile_pool(name="ps_st", bufs=2, space="PSUM"))
    # PV accumulator pairs (long-lived): dedicated 4x1 banks
    ps_po = ctx.enter_context(tc.tile_pool(name="ps_po", bufs=4, space="PSUM"))
    pt_pool = ctx.enter_context(tc.tile_pool(name="pt_pool", bufs=PEND + 2))
    rec_pool = ctx.enter_context(tc.tile_pool(name="rec_pool", bufs=4))
    rs_pool = ctx.enter_context(tc.tile_pool(name="rs_pool", bufs=2))
    osh_pool = ctx.enter_context(tc.tile_pool(name="osh_pool", bufs=2))
    y_pool = ctx.enter_context(tc.tile_pool(name="y_pool", bufs=3))

    # ---- resident SBUF tiles --------------------------------------------
    wq_sb = singles.tile([128, CCH, NH * HD], bf16, name="wq_sb")
    wk_sb = singles.tile([128, CCH, NH * HD], bf16, name="wk_sb")
    wv_sb = singles.tile([128, CCH, NH * HD], bf16, name="wv_sb")
    wp_sb = singles.tile([128, 2, C], bf16, name="wp_sb")
    tri_sb = singles.tile([128, 128], bf16, name="tri_sb")
    # xT per t-block: [128, cc, 512]
    xt_t = [singles.tile([128, CCH, QB], bf16, name=f"xt{i}")
            for i in range(N_QB)]
    # Q^T / K^T per (head pair, t-block): [128 = 2 heads x 64, 512]
    qt_t = [[singles.tile([128, QB], bf16, name=f"qt{p}_{i}")
             for i in range(N_QB)] for p in range(2)]
    kt_t = [[singles.tile([128, QB], bf16, name=f"kt{p}_{i}")
             for i in range(N_QB)] for p in range(2)]
    # V|ones per kc pair: [128 k, 2, head, 65]
    v_t = [singles.tile([128, 2, NH, HD + 1], bf16, name=f"v{j}")
           for j in range(N_KC // 2)]
    # O^T per (head pair, q-block)
    ot_t = [[singles.tile([128, QB], bf16, name=f"ot{p}_{i}")
             for i in range(N_QB)] for p in range(2)]

    # ---- input DMAs (wq first, xt0 in per-cc slices: first gen matmul can
    # start after ~wq + one slice instead of the whole 2.5MB prefix) -------
    nc.gpsimd.load_library(library_config.attn)
    nc.sync.dma_start(out=wq_sb, in_=wq.rearrange("(c p) n -> p c n", p=128))
    xtr_v = xtr.rearrange("p c (i q) -> p c i q", q=QB)
    for cc in range(CCH):
        nc.sync.dma_start(out=xt_t[0][:, cc, :], in_=xtr_v[:, cc, 0, :])
    nc.sync.dma_start(out=wk_sb, in_=wk.rearrange("(c p) n -> p c n", p=128))
    nc.sync.dma_start(out=wv_sb, in_=wv.rearrange("(c p) n -> p c n", p=128))
    for i in range(1, N_QB):
        nc.sync.dma_start(out=xt_t[i], in_=xtr[:, :, i * QB:(i + 1) * QB])
    nc.sync.dma_start(out=wp_sb, in_=wp.rearrange("(j p) n -> p j n", p=128))
    nc.sync.dma_start(out=tri_sb, in_=tri)
    for j in range(N_KC // 2):
        nc.vector.memset(v_t[j][:, :, :, HD:HD + 1], 1.0)

    # ---- group emitters (each = one PSUM-tile's worth of matmuls) ------
    def cast(eng, out, in_):
        if eng is nc.scalar:
            eng.copy(out, in_)
        else:
            eng.tensor_copy(out, in_)

    def gen_qkw(dst, w_sb, nb, eng):
        # wide: Q^T (or K^T) for BOTH head pairs of t-block nb
        pg = ps_st.tile([128, 2 * QB], f32, tag="st", name="pg_qk")
        for pr in range(2):
            for cc in range(CCH):
                nc.tensor.matmul(
                    pg[:, pr * QB:(pr + 1) * QB],
                    w_sb[:, cc, pr * 128:(pr + 1) * 128],
                    xt_t[nb][:, cc, :],
                    start=(cc == 0), stop=(cc == CCH - 1))
        for pr in range(2):
            cast(eng, dst[pr][nb], pg[:, pr * QB:(pr + 1) * QB])

    def gen_vw(nb, eng):
        # wide: V|ones for the 4 key chunks of t-block nb
        pg = ps_st.tile([128, 2 * QB], f32, tag="st", name="pg_v")
        for q in range(4):
            for cc in range(CCH):
                nc.tensor.matmul(
                    pg[:, q * 256:(q + 1) * 256],
                    xt_t[nb][:, cc, q * 128:(q + 1) * 128],
                    wv_sb[:, cc, :],
                    start=(cc == 0), stop=(cc == CCH - 1))
        for q in range(4):
            cast(
                eng, v_t[2 * nb + q // 2][:, q % 2, :, 0:HD],
                pg[:, q * 256:(q + 1) * 256].rearrange(
                    "p (h n) -> p h n", h=NH))
        if nb == 0:
            # emphasis: P column for k==0 gets exp(s+1); fold e into V|ones
            nc.scalar.mul(v_t[0][0:1, 0, :, :], v_t[0][0:1, 0, :, :],
                          float(math.exp(EMPHASIS)))

    def proj_w(qb, tci, eng):
        # wide: both 512-col halves of y rows [128*(4qb+tci) ..]
        tc_i = 4 * qb + tci
        py = ps_st.tile([128, 2 * QB], f32, tag="st", name="py_w")
        for ch in range(2):
            for pr2 in range(2):
                nc.tensor.matmul(
                    py[:, ch * QB:(ch + 1) * QB],
                    ot_t[pr2][qb][:, tci * 128:(tci + 1) * 128],
                    wp_sb[:, pr2, ch * QB:(ch + 1) * QB],
                    start=(pr2 == 0), stop=(pr2 == 1))
        ysb = y_pool.tile([128, C], f16, tag="ysb", name=f"ysb{tc_i}")
        cast(eng, ysb, py)
        nc.sync.dma_start(out=y[tc_i * 128:(tc_i + 1) * 128, :], in_=ysb)

    def proj_tail_wave(qb, tcis, eng0, eng1):
        # narrow groups from the (now idle) po pool; all pr2=0 matmuls are
        # emitted first so they run while the last head-pair is still being
        # normalized
        pys = {}
        for tci in tcis:
            for ch in range(2):
                py = pys[(tci, ch)] = ps_po.tile(
                    [128, QB], f32, tag="po", name=f"py_{tci}_{ch}")
                nc.tensor.matmul(
                    py, ot_t[0][qb][:, tci * 128:(tci + 1) * 128],
                    wp_sb[:, 0, ch * QB:(ch + 1) * QB],
                    start=True, stop=False)
        for tci in tcis:
            tc_i = 4 * qb + tci
            ysb = y_pool.tile([128, C], f16, tag="ysb", name=f"ysb{tc_i}")
            for ch, eng in ((0, eng0), (1, eng1)):
                nc.tensor.matmul(
                    pys[(tci, ch)],
                    ot_t[1][qb][:, tci * 128:(tci + 1) * 128],
                    wp_sb[:, 1, ch * QB:(ch + 1) * QB],
                    start=False, stop=True)
                cast(eng, ysb[:, ch * QB:(ch + 1) * QB], pys[(tci, ch)])
            nc.sync.dma_start(out=y[tc_i * 128:(tc_i + 1) * 128, :], in_=ysb)

    def gen_groups(nb):
        return [(gen_qkw, qt_t, wq_sb, nb), (gen_qkw, kt_t, wk_sb, nb),
                (gen_vw, nb)]

    def proj_groups(qb):
        return [(proj_w, qb, tci) for tci in range(4)]

    def run_group(g, eng):
        if g[0] is gen_qkw:
            gen_qkw(g[1], g[2], g[3], eng)
        elif g[0] is gen_vw:
            gen_vw(g[1], eng)
        else:
            proj_w(g[1], g[2], eng)

    # ---- attention ------------------------------------------------------
    def norm(pr, qb, s, po, last):
        # den row PSUM->SBUF on Pool, partition_broadcast on Pool (no DMA
        # machinery: the broadcast DMA cost ~6.5us issue-to-ready), then
        # 1/den in place on DVE (the custom-DVE write is only read by the
        # same-engine mul: custom-DVE writes are not cross-engine tracked),
        # then multiply.
        # (gpsimd cannot access PSUM: the den copy must ride DVE)
        rs = rs_pool.tile([1, QB], f32, tag="rs")
        nc.vector.tensor_copy(rs, po[HD:HD + 1, :])
        rec = rec_pool.tile([HD, QB], f32, tag="rec")
        nc.gpsimd.partition_broadcast(rec, rs)
        nc.vector.reciprocal_approx_fast(out=rec, in_=rec)
        if s == 0:
            nc.vector.tensor_mul(ot_t[pr][qb][0:HD, :], po[0:HD, :], rec)
        else:
            osh = osh_pool.tile([HD, QB], bf16, tag="osh")
            nc.vector.tensor_mul(osh, po[0:HD, :], rec)
            nc.sync.dma_start(out=ot_t[pr][qb][HD:128, :], in_=osh)

    def emit_pv(rec_):
        pr, qb, kc, w0, pt, po0, po1, nk = rec_
        v = v_t[kc // 2][:, kc % 2, :, :]
        nc.tensor.matmul(po0[0:HD + 1, w0:QB], v[:, 2 * pr, :],
                         pt[:, w0:QB],
                         start=(kc == 0), stop=(kc == nk - 1))
        nc.tensor.matmul(po1[0:HD + 1, w0:QB], v[:, 2 * pr + 1, :],
                         pt[:, QB + w0:2 * QB],
                         start=(kc == 0), stop=(kc == nk - 1))
        if kc == nk - 1:
            last = (qb == N_QB - 1)
            norm(pr, qb, 1, po1, last)
            norm(pr, qb, 0, po0, last)

    pending = []
    fillers = []
    f_emitted = 0

    def pace_fillers(i, nq, total):
        nonlocal f_emitted
        # front-load gen fillers (every other chunk from the start); proj
        # fillers spread out after the prev block's norms are emitted
        while f_emitted < total and fillers:
            g = fillers[0]
            is_proj = g[0] is proj_w
            due = sched_proj if is_proj else sched_gen
            if not due or i < due[0]:
                break
            due.pop(0)
            fillers.pop(0)
            run_group(g, filler_eng)
            f_emitted += 1

    # gen(0) runs as the prefix; casts on the then-idle ACT engine
    gen_qkw(qt_t, wq_sb, 0, nc.vector)
    gen_qkw(kt_t, wk_sb, 0, nc.vector)
    gen_vw(0, nc.vector)

    for qb in range(N_QB):
        nk = 4 * (qb + 1) if causal else N_KC
        nq = 2 * nk
        fillers = []
        if qb + 1 < N_QB:
            fillers += gen_groups(qb + 1)
        if qb >= 1:
            fillers += proj_groups(qb - 1)
        total = len(fillers)
        filler_eng = nc.vector
        n_gen = sum(1 for g in fillers if g[0] is not proj_w)
        n_proj = total - n_gen
        sched_gen = {0: [0, 1, 5], 1: [0, 2, 4], 2: [0, 3, 6], 3: []}[qb][:n_gen]
        sched_proj = {0: [], 1: [6, 9, 12, 15], 2: [8, 12, 16, 20],
                      3: [6, 12, 18, 24]}[qb][:n_proj]
        f_emitted = 0
        i = 0
        for pr in range(2):
            po0 = ps_po.tile([128, QB], f32, tag="po", name="po0")
            po1 = ps_po.tile([128, QB], f32, tag="po", name="po1")
            for kc in range(nk):
                r = kc - 4 * qb
                w0 = 128 * r if (causal and r > 0) else 0
                st = ps_st.tile([128, 2 * QB], f32, tag="st")
                for s in range(2):
                    r0, r1 = s * HD, (s + 1) * HD
                    nc.tensor.matmul(
                        st[:, s * QB + w0:(s + 1) * QB],
                        kt_t[pr][kc // 4][r0:r1, (kc % 4) * 128:(kc % 4 + 1) * 128],
                        qt_t[pr][qb][r0:r1, w0:QB],
                        start=True, stop=True)
                pt = pt_pool.tile([128, 2 * QB], bf16, tag="pt")
                if w0 == 0:
                    nc.scalar.activation(out=pt, in_=st, func=EXP)
                else:
                    stv = st.rearrange("p (a q) -> p a q", a=2)
                    ptv = pt.rearrange("p (a q) -> p a q", a=2)
                    nc.scalar.activation(out=ptv[:, :, w0:QB],
                                         in_=stv[:, :, w0:QB], func=EXP)
                if causal and r >= 0:
                    for s, meng in ((0, nc.vector), (1, nc.vector)):
                        meng.tensor_mul(
                            pt[:, s * QB + w0:s * QB + w0 + 128],
                            pt[:, s * QB + w0:s * QB + w0 + 128],
                            tri_sb)
                pending.append((pr, qb, kc, w0, pt, po0, po1, nk))
                while len(pending) > PEND:
                    emit_pv(pending.pop(0))
                if not BISECT_NOFILL:
                    pace_fillers(i, nq, total)
                i += 1
        if BISECT_NOFILL:
            while pending:
                emit_pv(pending.pop(0))
            while fillers:
                g = fillers.pop(0)
                run_group(g, filler_eng)
    while pending:
        emit_pv(pending.pop(0))
    # trailing projection of the last q-block: narrow groups from the now
    # idle po pool, casts alternating engines so consecutive groups pipeline
    for tcis in ((0, 1), (2, 3)):
        proj_tail_wave(N_QB - 1, tcis, nc.scalar, nc.vector)

    ctx.close()


def _prep_inputs(x, W_attn, W_proj, attn_mask):
    """Host-side shard + layout prep. Returns (in_maps, causal)."""
    bf = ml_dtypes.bfloat16
    causal = bool(np.array_equal(
        np.asarray(attn_mask),
        np.tril(np.ones((T, T), dtype=bool))))

    x = np.asarray(x, dtype=np.float32)
    Wa = np.asarray(W_attn, dtype=np.float32)
    Wp = np.asarray(W_proj, dtype=np.float32)

    scale = 1.0 / np.sqrt(np.float32(HD))
    # [128, cc, T]: partition p holds rows c = cc*128 + p of x[b].T
    xtr_b = [np.ascontiguousarray(
        x[b].T.reshape(CCH, 128, T).transpose(1, 0, 2)).astype(bf)
        for b in range(B)]

    # in-stripe causal triangle: tri[k, q] = 1.0 if k <= q else 0
    i = np.arange(128)
    tri = (i[:, None] <= i[None, :]).astype(bf)

    in_maps = []
    for core in range(N_CORES):
        b, h0 = core // 4, (core % 4) * NH
        hsl = slice(h0 * HD, (h0 + NH) * HD)
        wq_c = np.ascontiguousarray(Wa[:, hsl] * scale).astype(bf)
        wk_c = np.ascontiguousarray(Wa[:, C + h0 * HD: C + (h0 + NH) * HD]).astype(bf)
        wv_c = np.ascontiguousarray(Wa[:, 2 * C + h0 * HD: 2 * C + (h0 + NH) * HD]).astype(bf)
        wp_c = np.ascontiguousarray(Wp[hsl, :]).astype(bf)
        in_maps.append({
            "xtr": xtr_b[b], "wq": wq_c, "wk": wk_c, "wv": wv_c,
            "wp": wp_c, "tri": tri,
        })
    return in_maps, causal


def kernel(x, W_attn, W_proj, attn_mask, _trace=False):
    from concourse import bass_utils

    in_maps, causal = _prep_inputs(x, W_attn, W_proj, attn_mask)
    key = ("causal" if causal else "dense")
    if key not in _COMPILED:
        _COMPILED[key] = _build(causal)
    nc = _COMPILED[key]

    res = bass_utils.run_bass_kernel_spmd(
        nc, in_maps, core_ids=list(range(N_CORES)), trace=_trace)

    y = np.zeros((B, T, C), dtype=np.float32)
    for core in range(N_CORES):
        y[core // 4] += res.results[core]["y"].astype(np.float32)
    if _trace:
        kernel._last_results = res
    return y

